# revision 39
# baseline (speedup 1.0000x reference)
"""nn_Encoder_22316650070699: 6-layer post-LN transformer encoder on 8 TRN2
NeuronCores, data-parallel over the batch (one sequence per core).

kernel(**inputs) takes the FULL unsharded inputs (as from setup_inputs()) and
returns the FULL (8, 1024, 768) fp32 output.

Feature-major design (v2): the residual stream lives feature-major
([128 feat, 1024 tok] x 6 chunks) for the whole layer, so the only PE
transposes are one phase at the start (embedding -> fm) and one at the end
(fm -> token-major output).  Per layer:
  - Q/K projections run at M=128 (full PE width) into 6 tiles [128, 1024];
    partition-swapped copies (SBUF->SBUF DMA) keep the per-head
    .view()-gather strips partition-aligned for the vector engine.
  - Scores for a head pair run row-packed (head A on PE rows 0-63, head B on
    rows 64-127 via base-partition-derived tile_position) -> 2x throughput.
  - exp() on ScalarE reads the packed [128, 1024] score tiles (A|B halves).
  - AV uses the ones-column trick for the softmax denominator; the attention
    output is assembled feature-major (no transposes), normalized with a
    reciprocal + gpsimd partition_broadcast + DVE multiply.
  - LayerNorm is feature-major: sum/sum-of-squares via ones-lhsT matmuls
    (partition reduction on the PE), stats broadcast via gpsimd, affine
    applied with per-partition gamma/beta.
  - FFN1/FFN2 run fully feature-major at M=K=128, biases folded into the
    per-partition activation evictions.
"""

import numpy as np
import ml_dtypes
from contextlib import ExitStack

F32 = None  # set in _lazy_imports
_BASS = {}


def _lazy_imports():
    global F32
    if _BASS:
        return
    import concourse.bass as bass
    import concourse.tile as tile
    from concourse import bacc, mybir, library_config
    from concourse.masks import make_identity
    _BASS.update(bass=bass, tile=tile, bacc=bacc, mybir=mybir,
                 make_identity=make_identity, library_config=library_config)
    F32 = mybir.dt.float32


B, S, D, H, DF, L, V, MAXLEN = 8, 1024, 768, 12, 3072, 6, 32000, 2048
HD = D // H
P = 128
NT = S // P      # 8 token tiles / k-tiles
NF = D // P      # 6 feature chunks
NDF = DF // P    # 24
KT = S // P
EPS = 1e-5
N_CORES = 8


def _build_encoder(L_layers=L, n_cores=N_CORES, debug_stage=None):
    _lazy_imports()
    bass = _BASS["bass"]
    tile = _BASS["tile"]
    bacc = _BASS["bacc"]
    mybir = _BASS["mybir"]
    make_identity = _BASS["make_identity"]
    F32 = mybir.dt.float32
    BF16 = mybir.dt.bfloat16
    I32 = mybir.dt.int32
    AF = mybir.ActivationFunctionType
    OP = mybir.AluOpType

    nc = bacc.Bacc("TRN2", target_bir_lowering=False, debug=False,
                   num_devices=n_cores)

    ids = nc.dram_tensor("ids", [S, 1], I32, kind="ExternalInput")
    emb = nc.dram_tensor("emb", [V, D], F32, kind="ExternalInput")
    pe = nc.dram_tensor("pe", [S, D], F32, kind="ExternalInput")
    wqT = nc.dram_tensor("wqT", [L_layers, D, D], BF16, kind="ExternalInput")
    wkT = nc.dram_tensor("wkT", [L_layers, D, D], BF16, kind="ExternalInput")
    wvT = nc.dram_tensor("wvT", [L_layers, D, D], BF16, kind="ExternalInput")
    bqkvr = nc.dram_tensor("bqkvr", [L_layers, 3, 1, D], BF16,
                           kind="ExternalInput")
    w1T = nc.dram_tensor("w1T", [L_layers, D, DF], BF16, kind="ExternalInput")
    b1c = nc.dram_tensor("b1c", [L_layers, P, NDF], F32, kind="ExternalInput")
    w2T = nc.dram_tensor("w2T", [L_layers, DF, D], BF16, kind="ExternalInput")
    b2c = nc.dram_tensor("b2c", [L_layers, P, NF], F32, kind="ExternalInput")
    lng1 = nc.dram_tensor("lng1", [L_layers, P, NF], F32, kind="ExternalInput")
    lnb1 = nc.dram_tensor("lnb1", [L_layers, P, NF], F32, kind="ExternalInput")
    lng2 = nc.dram_tensor("lng2", [L_layers, P, NF], F32, kind="ExternalInput")
    lnb2 = nc.dram_tensor("lnb2", [L_layers, P, NF], F32, kind="ExternalInput")
    selc = nc.dram_tensor("selc", [3, 2, P], F32, kind="ExternalInput")
    out = nc.dram_tensor("out", [S, D], F32, kind="ExternalOutput")

    with tile.TileContext(nc) as tc, ExitStack() as ctx:
        # ---- pools --------------------------------------------------------
        xp = ctx.enter_context(tc.tile_pool(name="xp", bufs=6))      # [128,1024] f32 residual master
        xbp = ctx.enter_context(tc.tile_pool(name="xbp", bufs=6))    # [128,1024] bf16 matmul copy of x
        bfp = ctx.enter_context(tc.tile_pool(name="bfp", bufs=30))   # [128,1024] bf16 shared ring
        qtp = ctx.enter_context(tc.tile_pool(name="qtp", bufs=4))    # [128,1032] bf16 qT/kT pairs
        ptp = ctx.enter_context(tc.tile_pool(name="ptp", bufs=3))    # [128,1024] bf16 exp(scores)
        vap = ctx.enter_context(tc.tile_pool(name="vap", bufs=4))    # [128,520] bf16 v-aug
        f32s = ctx.enter_context(tc.tile_pool(name="f32s", bufs=3))  # f32 scratch (tags)
        wqp = ctx.enter_context(tc.tile_pool(name="wqp", bufs=8))    # [128,768] bf16 qkv weights
        w1p = ctx.enter_context(tc.tile_pool(name="w1p", bufs=6))    # [128,1536] bf16 ffn1 weights
        w2p = ctx.enter_context(tc.tile_pool(name="w2p", bufs=48))   # [128,128] bf16 ffn2 weight chunks
        smp = ctx.enter_context(tc.tile_pool(name="smp", bufs=8))    # small tiles
        cst = ctx.enter_context(tc.tile_pool(name="cst", bufs=1))
        drp = ctx.enter_context(tc.tile_pool(name="drp", bufs=1, space="DRAM"))

        ps_big = ctx.enter_context(tc.tile_pool(name="ps_big", bufs=2, space="PSUM"))
        ps_av = ctx.enter_context(tc.tile_pool(name="ps_av", bufs=2, space="PSUM"))

        # ---- constants ----------------------------------------------------
        idb = cst.tile([P, P], BF16)
        make_identity(nc, idb)
        ones_row = cst.tile([1, P], BF16)
        nc.vector.memset(ones_row[:], 1.0)
        ones_col = cst.tile([P, 1], BF16)
        nc.vector.memset(ones_col[:], 1.0)
        eps_c = cst.tile([P, 1], F32)
        nc.vector.memset(eps_c[:], EPS)
        # broadcast helpers: out[128, N] = sel.T @ rows  (PE-based broadcast)
        sel2 = cst.tile([2, P], F32)
        nc.sync.dma_start(sel2[:], selc[0])
        selr0 = cst.tile([2, P], F32)
        nc.sync.dma_start(selr0[:], selc[1])
        selr1 = cst.tile([2, P], F32)
        nc.sync.dma_start(selr1[:], selc[2])

        # DRAM scratch for q/k/v in flat (S*12, 64) layout: head h of the
        # torch .view() is rows [1024h, 1024h+1024) of the flat matrix.
        qkv_dram = drp.tile([3, S * H, HD], BF16)

        # ---- embedding: x = emb[ids] + pe, transposed to feature-major ----
        x = [xp.tile([P, S], F32, tag="x", name=f"x{F}") for F in range(NF)]
        xbf = [xbp.tile([P, S], BF16, tag="xbf", name=f"xbf{F}")
               for F in range(NF)]
        for T in range(NT):
            idt = smp.tile([P, 1], I32, tag="idt")
            nc.sync.dma_start(idt[:], ids[P * T:P * (T + 1), :])
            g = f32s.tile([P, D], F32, tag="sb")
            nc.gpsimd.indirect_dma_start(
                out=g[:], out_offset=None, in_=emb[:],
                in_offset=bass.IndirectOffsetOnAxis(ap=idt[:, :1], axis=0))
            pet = f32s.tile([P, D], F32, tag="sb")
            nc.sync.dma_start(pet[:], pe[P * T:P * (T + 1), :])
            xt = f32s.tile([P, D], F32, tag="sb")
            nc.vector.tensor_add(xt[:], g[:], pet[:])
            xtb = bfp.tile([P, D], BF16, tag="bf")
            nc.scalar.copy(xtb[:], xt[:])
            for F in range(NF):
                tr = ps_big.tile([P, P], BF16, tag="big")
                nc.tensor.transpose(tr[:], xtb[:, P * F:P * (F + 1)], idb[:])
                nc.vector.tensor_copy(xbf[F][:, P * T:P * (T + 1)], tr[:])
        for F in range(NF):
            nc.vector.tensor_copy(x[F][:], xbf[F][:])

        def dump_fm(tiles, stage):
            """Debug: DMA up to 6 [128, <=1024] tiles into `out` (flat)."""
            if debug_stage != stage:
                return
            flat = out[:].rearrange("s d -> (s d)").rearrange(
                "(f p t) -> f p t", f=6, p=P)
            for i, t in enumerate(tiles[:6]):
                tt = t
                if t.dtype != F32:
                    ft = f32s.tile([P, S], F32, tag="dump", bufs=2)
                    nc.vector.tensor_copy(ft[0:t.shape[0], 0:t.free_size()],
                                          t[:])
                    tt = ft[0:t.shape[0], 0:t.free_size()]
                nc.sync.dma_start(flat[i, 0:tt.shape[0], 0:1024]
                                  if tt.free_size() >= 1024 else
                                  flat[i, 0:tt.shape[0], 0:tt.free_size()],
                                  tt[:, 0:1024] if tt.free_size() >= 1024
                                  else tt[:])

        dump_fm(x, "embed")

        for l in range(L_layers):
            # ---- per-layer consts ----------------------------------------
            g1t = smp.tile([P, NF], F32, tag="ln")
            nc.sync.dma_start(g1t[:], lng1[l])
            b1t = smp.tile([P, NF], F32, tag="ln")
            nc.sync.dma_start(b1t[:], lnb1[l])
            g2t = smp.tile([P, NF], F32, tag="ln")
            nc.sync.dma_start(g2t[:], lng2[l])
            b2t = smp.tile([P, NF], F32, tag="ln")
            nc.sync.dma_start(b2t[:], lnb2[l])
            b1ct = smp.tile([P, NDF], F32, tag="b1")
            nc.sync.dma_start(b1ct[:], b1c[l])
            b2ct = smp.tile([P, NF], F32, tag="b2")
            nc.sync.dma_start(b2ct[:], b2c[l])

            # ---- Q/K/V projections, token-major -> DRAM flat -------------
            for ti, wT in enumerate((wqT, wkT, wvT)):
                wsb = []
                for F in range(NF):
                    w = wqp.tile([P, D], BF16, tag="w")
                    nc.sync.dma_start(w[:], wT[l, P * F:P * (F + 1), :])
                    wsb.append(w)
                brow = smp.tile([1, D], BF16, tag="brow", bufs=3)
                nc.sync.dma_start(brow[:], bqkvr[l, ti])
                for T in range(NT):
                    ps = ps_big.tile([P, S], F32, tag="big")
                    for nb, (n0, n1) in enumerate(((0, 512), (512, 768))):
                        for F in range(NF):
                            nc.tensor.matmul(
                                ps[:, n0:n1],
                                lhsT=xbf[F][:, P * T:P * (T + 1)],
                                rhs=wsb[F][:, n0:n1],
                                start=(F == 0), stop=False)
                        nc.tensor.matmul(
                            ps[:, n0:n1], lhsT=ones_row[:, :],
                            rhs=brow[:, n0:n1], start=False, stop=True)
                    ev = bfp.tile([P, D], BF16, tag="bf")
                    nc.vector.tensor_copy(ev[:], ps[:, :D])
                    nc.sync.dma_start(
                        qkv_dram[ti, H * P * T: H * P * (T + 1), :]
                        .rearrange("(p a) d -> p a d", p=P),
                        ev[:].rearrange("p (a d) -> p a d", d=HD))

            # ---- attention, head pairs (A=2p at part 0-63, B at 64-127) --
            for p in range(H // 2):
                hA, hB = 2 * p, 2 * p + 1
                va2 = []
                for h in (hA, hB):
                    va = vap.tile([P, KT * (HD + 1)], BF16, tag="va")
                    nc.sync.dma_start(
                        va[:].rearrange("p (k d) -> p k d", d=HD + 1)[:, :, 0:HD],
                        qkv_dram[2, S * h:S * (h + 1), :]
                        .rearrange("(k p) d -> p k d", p=P))
                    nc.vector.memset(
                        va[:].rearrange("p (k d) -> p k d", d=HD + 1)
                        [:, :, HD:HD + 1], 1.0)
                    va2.append(va)

                # qT/kT [128, 1024] for the pair via DMA transpose: head h is
                # rows [1024h, 1024h+1024) of the flat (S*12, 64) matrix.
                qT = qtp.tile([HD * 2, S], BF16, tag="qt")
                kTt = qtp.tile([HD * 2, S], BF16, tag="qt")
                for ti, dst in ((0, qT), (1, kTt)):
                    for half, h in ((0, hA), (1, hB)):
                        hs = slice(HD * half, HD * (half + 1))
                        nc.sync.dma_start_transpose(
                            dst[hs, :], qkv_dram[ti, S * h:S * (h + 1), :])
                avA = ps_av.tile([HD + 1, S], F32, tag="av")
                avB = ps_av.tile([HD + 1, S], F32, tag="av")
                for k in range(KT):
                    for nb in range(2):
                        sc = ps_big.tile([P, S], F32, tag="big")
                        nc.tensor.matmul(
                            sc[:, 0:512],
                            lhsT=kTt[0:HD, P * k:P * (k + 1)],
                            rhs=qT[0:HD, 512 * nb:512 * (nb + 1)],
                            start=True, stop=True)
                        nc.tensor.matmul(
                            sc[:, 512:1024],
                            lhsT=kTt[HD:P, P * k:P * (k + 1)],
                            rhs=qT[HD:P, 512 * nb:512 * (nb + 1)],
                            start=True, stop=True)
                        pt = ptp.tile([P, S], BF16, tag="pt")
                        nc.scalar.activation(pt[:], sc[:], AF.Exp,
                                             scale=1.0 / 8.0)
                        if l == 0 and p == 0 and k == 0 and nb == 0:
                            dump_fm([pt], "exp00")
                        nc.tensor.matmul(
                            avA[:, 512 * nb:512 * (nb + 1)],
                            lhsT=va2[0][:, (HD + 1) * k:(HD + 1) * (k + 1)],
                            rhs=pt[:, 0:512],
                            start=(k == 0), stop=(k == KT - 1))
                        nc.tensor.matmul(
                            avB[:, 512 * nb:512 * (nb + 1)],
                            lhsT=va2[1][:, (HD + 1) * k:(HD + 1) * (k + 1)],
                            rhs=pt[:, 512:1024],
                            start=(k == 0), stop=(k == KT - 1))

                # normalize and accumulate into the residual, feature-major:
                # head A -> x[p][0:64], head B -> x[p][64:128].
                sbA = f32s.tile([HD + 1, S], F32, tag="sb")
                nc.vector.tensor_copy(sbA[:], avA[:])
                sbB = f32s.tile([HD + 1, S], F32, tag="sb")
                nc.vector.tensor_copy(sbB[:], avB[:])
                nc.vector.reciprocal(sbA[HD:HD + 1, :], sbA[HD:HD + 1, :])
                nc.vector.reciprocal(sbB[HD:HD + 1, :], sbB[HD:HD + 1, :])
                sbBsh = f32s.tile([P, S], F32, tag="sb")
                nc.sync.dma_start(sbBsh[HD:P, :], sbB[0:HD, :])
                recs = f32s.tile([2, S], F32, tag="rc", bufs=2)
                nc.sync.dma_start(recs[0:1, :], sbA[HD:HD + 1, :])
                nc.sync.dma_start(recs[1:2, :], sbB[HD:HD + 1, :])
                bk = ps_big.tile([P, S], F32, tag="big")
                for nb in range(2):
                    nc.tensor.matmul(bk[:, 512 * nb:512 * (nb + 1)],
                                     lhsT=sel2[:, :],
                                     rhs=recs[:, 512 * nb:512 * (nb + 1)],
                                     start=True, stop=True)
                if l == 0 and p == 0:
                    dump_fm([sbA, sbB, bk, sbBsh], "norm0")
                nc.vector.tensor_tensor(sbA[0:HD, :], sbA[0:HD, :],
                                        bk[0:HD, :], op=OP.mult)
                nc.vector.tensor_tensor(sbBsh[HD:P, :], sbBsh[HD:P, :],
                                        bk[HD:P, :], op=OP.mult)
                nc.vector.tensor_add(x[p][0:HD, :], x[p][0:HD, :],
                                     sbA[0:HD, :])
                nc.vector.tensor_add(x[p][HD:P, :], x[p][HD:P, :],
                                     sbBsh[HD:P, :])
            if l == 0:
                dump_fm(x, "attn")

            # ---- feature-major LayerNorm (in place on x) -----------------
            def layernorm_fm(g_t, b_t, out_tag, out_name):
                st = ps_av.tile([33, S], F32, tag="av")
                for F in range(NF):
                    rb = bfp.tile([P, S], BF16, tag="bf")
                    nc.vector.tensor_copy(rb[:], x[F][:])
                    sq = bfp.tile([P, S], BF16, tag="bf")
                    nc.vector.tensor_tensor(sq[:], x[F][:], x[F][:],
                                            op=OP.mult)
                    for nb in range(2):
                        nc.tensor.matmul(
                            st[0:1, 512 * nb:512 * (nb + 1)],
                            lhsT=ones_col[:, :],
                            rhs=rb[:, 512 * nb:512 * (nb + 1)],
                            start=(F == 0), stop=(F == NF - 1))
                        nc.tensor.matmul(
                            st[32:33, 512 * nb:512 * (nb + 1)],
                            lhsT=ones_col[:, :],
                            rhs=sq[:, 512 * nb:512 * (nb + 1)],
                            start=(F == 0), stop=(F == NF - 1))
                sts = f32s.tile([33, S], F32, tag="sb")
                nc.vector.tensor_copy(sts[32:33, :], st[32:33, :])
                strows = f32s.tile([2, S], F32, tag="rc", bufs=2)
                nc.vector.tensor_copy(strows[0:1, :], st[0:1, :])
                nc.sync.dma_start(strows[1:2, :], sts[32:33, :])
                mS_ps = ps_big.tile([P, S], F32, tag="big")
                qS_ps = ps_big.tile([P, S], F32, tag="big")
                for nb in range(2):
                    nc.tensor.matmul(mS_ps[:, 512 * nb:512 * (nb + 1)],
                                     lhsT=selr0[:, :],
                                     rhs=strows[:, 512 * nb:512 * (nb + 1)],
                                     start=True, stop=True)
                    nc.tensor.matmul(qS_ps[:, 512 * nb:512 * (nb + 1)],
                                     lhsT=selr1[:, :],
                                     rhs=strows[:, 512 * nb:512 * (nb + 1)],
                                     start=True, stop=True)
                # mean, var, rstd, -mean*rstd  (all [128, S] broadcast tiles)
                mS = f32s.tile([P, S], F32, tag="bk", bufs=2)
                nc.vector.tensor_scalar_mul(mS[:], mS_ps[:], 1.0 / D)
                qS = f32s.tile([P, S], F32, tag="lns")
                nc.vector.tensor_scalar_mul(qS[:], qS_ps[:], 1.0 / D)
                msq = f32s.tile([P, S], F32, tag="lns")
                nc.vector.tensor_tensor(msq[:], mS[:], mS[:], op=OP.mult)
                nc.vector.tensor_tensor(qS[:], qS[:], msq[:], op=OP.subtract)
                sd = msq  # reuse storage
                nc.scalar.activation(sd[:], qS[:], AF.Sqrt, bias=eps_c[:, :1])
                rstd = qS  # reuse storage
                nc.vector.reciprocal(rstd[:], sd[:])
                nmr = f32s.tile([P, S], F32, tag="lns")
                nc.vector.tensor_tensor(nmr[:], mS[:], rstd[:], op=OP.mult)
                nc.vector.tensor_scalar_mul(nmr[:], nmr[:], -1.0)
                outs = []
                for F in range(NF):
                    cF = f32s.tile([P, S], F32, tag="sb")
                    nc.vector.tensor_scalar(cF[:], nmr[:], g_t[:, F:F + 1],
                                            b_t[:, F:F + 1],
                                            op0=OP.mult, op1=OP.add)
                    t = f32s.tile([P, S], F32, tag="sb")
                    nc.vector.tensor_tensor(t[:], x[F][:], rstd[:], op=OP.mult)
                    nc.vector.scalar_tensor_tensor(
                        x[F][:], t[:], g_t[:, F:F + 1], cF[:],
                        op0=OP.mult, op1=OP.add)
                    ob = (bfp.tile([P, S], BF16, tag="bf",
                                   name=f"{out_name}{F}")
                          if out_tag == "bf" else
                          xbp.tile([P, S], BF16, tag="xbf",
                                   name=f"{out_name}{F}"))
                    nc.vector.tensor_copy(ob[:], x[F][:])
                    outs.append(ob)
                return outs

            y1bf = layernorm_fm(g1t, b1t, "bf", "y1bf")
            if l == 0:
                dump_fm(x, "ln1")

            # ---- FFN (feature-major) -------------------------------------
            hT = []
            for half in range(2):
                w1sb = []
                for F in range(NF):
                    w = w1p.tile([P, DF // 2], BF16, tag="w1")
                    nc.sync.dma_start(
                        w[:], w1T[l, P * F:P * (F + 1),
                                  (DF // 2) * half:(DF // 2) * (half + 1)])
                    w1sb.append(w)
                for ci in range(NDF // 2):
                    c = (NDF // 2) * half + ci
                    ps = ps_big.tile([P, S], F32, tag="big")
                    for F in range(NF):
                        for nb in range(2):
                            nc.tensor.matmul(
                                ps[:, 512 * nb:512 * (nb + 1)],
                                lhsT=w1sb[F][:, P * ci:P * (ci + 1)],
                                rhs=y1bf[F][:, 512 * nb:512 * (nb + 1)],
                                start=(F == 0), stop=(F == NF - 1))
                    ht = bfp.tile([P, S], BF16, tag="bf", name=f"ht{c}")
                    nc.scalar.activation(ht[:], ps[:], AF.Relu,
                                         bias=b1ct[:, c:c + 1])
                    hT.append(ht)
            for j in range(NF):
                w2sb = []
                for c in range(NDF):
                    w = w2p.tile([P, P], BF16, tag="w2")
                    nc.sync.dma_start(
                        w[:], w2T[l, P * c:P * (c + 1), P * j:P * (j + 1)])
                    w2sb.append(w)
                ps = ps_big.tile([P, S], F32, tag="big")
                for c in range(NDF):
                    for nb in range(2):
                        nc.tensor.matmul(
                            ps[:, 512 * nb:512 * (nb + 1)],
                            lhsT=w2sb[c][:, :],
                            rhs=hT[c][:, 512 * nb:512 * (nb + 1)],
                            start=(c == 0), stop=(c == NDF - 1))
                f2 = f32s.tile([P, S], F32, tag="sb")
                nc.vector.tensor_scalar_add(f2[:], ps[:], b2ct[:, j:j + 1])
                nc.vector.tensor_add(x[j][:], x[j][:], f2[:])
            if l == 0:
                dump_fm(x, "ffn")

            # ---- LN2 -> x (and bf16 copies become next layer's xbf) ------
            xbf = layernorm_fm(g2t, b2t, "xbf", "xnbf")

        # ---- output: fm -> token-major ------------------------------------
        for F in range(NF if debug_stage is None else 0):
            for T in range(NT):
                tr = ps_big.tile([P, P], BF16, tag="big")
                nc.tensor.transpose(tr[:], xbf[F][:, P * T:P * (T + 1)],
                                    idb[:])
                of = f32s.tile([P, P], F32, tag="sb")
                nc.vector.tensor_copy(of[:], tr[:])
                nc.sync.dma_start(out[P * T:P * (T + 1), P * F:P * (F + 1)],
                                  of[:])

    nc.compile()
    return nc


def _selc():
    s = np.zeros((3, 2, P), np.float32)
    s[0, 0, 0:HD] = 1.0   # sel2: head-A rows from stat row 0
    s[0, 1, HD:P] = 1.0   # sel2: head-B rows from stat row 1
    s[1, 0, :] = 1.0      # selr0: broadcast row 0
    s[2, 1, :] = 1.0      # selr1: broadcast row 1
    return s


def _prep_in_maps(inputs):
    bf = ml_dtypes.bfloat16
    Lw = np.asarray(inputs["Wq"]).shape[0]

    def fmc(v):  # [L, D] -> [L, P, NF] feature-chunk-major
        return np.ascontiguousarray(
            np.asarray(v, np.float32).reshape(Lw, NF, P).transpose(0, 2, 1))

    shared = {
        "emb": np.ascontiguousarray(np.asarray(inputs["emb"], np.float32)),
        "pe": np.ascontiguousarray(np.asarray(inputs["pe"], np.float32)[:S]),
        "wqT": np.ascontiguousarray(
            np.asarray(inputs["Wq"]).transpose(0, 2, 1)).astype(bf),
        "wkT": np.ascontiguousarray(
            np.asarray(inputs["Wk"]).transpose(0, 2, 1)).astype(bf),
        "wvT": np.ascontiguousarray(
            np.asarray(inputs["Wv"]).transpose(0, 2, 1)).astype(bf),
        "bqkvr": np.ascontiguousarray(np.stack(
            [np.asarray(inputs["bq"]), np.asarray(inputs["bk"]),
             np.asarray(inputs["bv"])], axis=1)
            .reshape(Lw, 3, 1, D)).astype(bf),
        "w1T": np.ascontiguousarray(
            np.asarray(inputs["W1"]).transpose(0, 2, 1)).astype(bf),
        "b1c": np.ascontiguousarray(
            np.asarray(inputs["b1"], np.float32).reshape(Lw, NDF, P)
            .transpose(0, 2, 1)),
        "w2T": np.ascontiguousarray(
            np.asarray(inputs["W2"]).transpose(0, 2, 1)).astype(bf),
        "b2c": fmc(inputs["b2"]),
        "lng1": fmc(inputs["ln1_g"]),
        "lnb1": fmc(inputs["ln1_b"]),
        "lng2": fmc(inputs["ln2_g"]),
        "lnb2": fmc(inputs["ln2_b"]),
        "selc": _selc(),
    }
    ids_all = np.asarray(inputs["input_ids"]).astype(np.int32)
    in_maps = []
    for c in range(N_CORES):
        m = dict(shared)
        m["ids"] = np.ascontiguousarray(ids_all[c].reshape(S, 1))
        in_maps.append(m)
    return in_maps


_CACHE = {}


def _get_runner():
    """Build (once) the compiled NEFF wrapped in a sharded PJRT callable."""
    if "run" in _CACHE:
        return _CACHE["run"]
    _lazy_imports()
    mybir = _BASS["mybir"]
    import jax
    import jax.numpy as jnp
    from jax.sharding import Mesh, PartitionSpec
    from jax.experimental.shard_map import shard_map
    from concourse import bass2jax
    from concourse.bass2jax import _bass_exec_p, partition_id_tensor

    nc = _build_encoder()
    bass2jax.install_neuronx_cc_hook()

    in_names, out_names, out_avals, zero_outs = [], [], [], []
    partition_name = (nc.partition_id_tensor.name
                      if nc.partition_id_tensor else None)
    for alloc in nc.m.functions[0].allocations:
        if not isinstance(alloc, mybir.MemoryLocationSet):
            continue
        name = alloc.memorylocations[0].name
        if alloc.kind == "ExternalInput":
            if name != partition_name:
                in_names.append(name)
        elif alloc.kind == "ExternalOutput":
            out_names.append(name)
            shape = tuple(alloc.tensor_shape)
            dtype = mybir.dt.np(alloc.dtype)
            out_avals.append(jax.core.ShapedArray(shape, dtype))
            zero_outs.append(np.zeros(shape, dtype))
    n_params = len(in_names)
    n_outs = len(out_avals)
    all_in_names = list(in_names) + list(out_names)
    if partition_name is not None:
        all_in_names.append(partition_name)
    donate = tuple(range(n_params, n_params + n_outs))

    def _body(*args):
        operands = list(args)
        if partition_name is not None:
            operands.append(partition_id_tensor())
        outs = _bass_exec_p.bind(
            *operands,
            out_avals=tuple(out_avals),
            in_names=tuple(all_in_names),
            out_names=tuple(out_names),
            lowering_input_output_aliases=(),
            sim_require_finite=True,
            sim_require_nnan=True,
            nc=nc,
        )
        return tuple(outs)

    devices = jax.devices()[:N_CORES]
    mesh = Mesh(np.asarray(devices), ("core",))
    in_specs = (PartitionSpec("core"),) * (n_params + n_outs)
    out_specs = (PartitionSpec("core"),) * n_outs
    sharded = jax.jit(
        shard_map(_body, mesh=mesh, in_specs=in_specs, out_specs=out_specs,
                  check_rep=False),
        donate_argnums=donate, keep_unused=True)

    def run(in_maps, timing_iters=0):
        concat_in = [
            np.concatenate([np.asarray(in_maps[c][k]) for c in range(N_CORES)],
                           axis=0)
            for k in in_names
        ]
        concat_zeros = [
            np.zeros((N_CORES * z.shape[0], *z.shape[1:]), z.dtype)
            for z in zero_outs
        ]
        out_arrs = sharded(*concat_in, *concat_zeros)
        results = [
            {name: np.asarray(out_arrs[i]).reshape(
                N_CORES, *out_avals[i].shape)[c]
             for i, name in enumerate(out_names)}
            for c in range(N_CORES)
        ]
        times = []
        if timing_iters:
            import time
            stage = jax.jit(shard_map(
                lambda *xs: xs, mesh=mesh,
                in_specs=(PartitionSpec("core"),) * len(concat_in),
                out_specs=(PartitionSpec("core"),) * len(concat_in),
                check_rep=False))
            dev_in = stage(*concat_in)
            jax.block_until_ready(dev_in)
            for _ in range(timing_iters):
                zs = [z.copy() for z in concat_zeros]
                t0 = time.perf_counter()
                o = sharded(*dev_in, *zs)
                jax.block_until_ready(o)
                times.append(time.perf_counter() - t0)
        return results, times

    _CACHE["run"] = run
    return run


def kernel(**inputs):
    run = _get_runner()
    in_maps = _prep_in_maps(inputs)
    results, _ = run(in_maps)
    out = np.stack([results[c]["out"] for c in range(N_CORES)], axis=0)
    return out


# revision 41
# speedup vs baseline: 3.7349x; 3.7349x over previous
"""nn_Encoder_22316650070699: 6-layer post-LN transformer encoder on 8 TRN2
NeuronCores, data-parallel over the batch (one sequence per core).

kernel(**inputs) takes the FULL unsharded inputs (as from setup_inputs()) and
returns the FULL (8, 1024, 768) fp32 output.

Feature-major design (v2): the residual stream lives feature-major
([128 feat, 1024 tok] x 6 chunks) for the whole layer, so the only PE
transposes are one phase at the start (embedding -> fm) and one at the end
(fm -> token-major output).  Per layer:
  - Q/K projections run at M=128 (full PE width) into 6 tiles [128, 1024];
    partition-swapped copies (SBUF->SBUF DMA) keep the per-head
    .view()-gather strips partition-aligned for the vector engine.
  - Scores for a head pair run row-packed (head A on PE rows 0-63, head B on
    rows 64-127 via base-partition-derived tile_position) -> 2x throughput.
  - exp() on ScalarE reads the packed [128, 1024] score tiles (A|B halves).
  - AV uses the ones-column trick for the softmax denominator; the attention
    output is assembled feature-major (no transposes), normalized with a
    reciprocal + gpsimd partition_broadcast + DVE multiply.
  - LayerNorm is feature-major: sum/sum-of-squares via ones-lhsT matmuls
    (partition reduction on the PE), stats broadcast via gpsimd, affine
    applied with per-partition gamma/beta.
  - FFN1/FFN2 run fully feature-major at M=K=128, biases folded into the
    per-partition activation evictions.
"""

import numpy as np
import ml_dtypes
from contextlib import ExitStack

F32 = None  # set in _lazy_imports
_BASS = {}


def _lazy_imports():
    global F32
    if _BASS:
        return
    import concourse.bass as bass
    import concourse.tile as tile
    from concourse import bacc, mybir, library_config
    from concourse.masks import make_identity
    _BASS.update(bass=bass, tile=tile, bacc=bacc, mybir=mybir,
                 make_identity=make_identity, library_config=library_config)
    F32 = mybir.dt.float32


B, S, D, H, DF, L, V, MAXLEN = 8, 1024, 768, 12, 3072, 6, 32000, 2048
HD = D // H
P = 128
NT = S // P      # 8 token tiles / k-tiles
NF = D // P      # 6 feature chunks
NDF = DF // P    # 24
KT = S // P
EPS = 1e-5
N_CORES = 8


def _build_encoder(L_layers=L, n_cores=N_CORES, debug_stage=None):
    _lazy_imports()
    bass = _BASS["bass"]
    tile = _BASS["tile"]
    bacc = _BASS["bacc"]
    mybir = _BASS["mybir"]
    make_identity = _BASS["make_identity"]
    F32 = mybir.dt.float32
    BF16 = mybir.dt.bfloat16
    I32 = mybir.dt.int32
    AF = mybir.ActivationFunctionType
    OP = mybir.AluOpType

    nc = bacc.Bacc("TRN2", target_bir_lowering=False, debug=False,
                   num_devices=n_cores)

    ids = nc.dram_tensor("ids", [S, 1], I32, kind="ExternalInput")
    emb = nc.dram_tensor("emb", [V, D], F32, kind="ExternalInput")
    pe = nc.dram_tensor("pe", [S, D], F32, kind="ExternalInput")
    wqT = nc.dram_tensor("wqT", [L_layers, D, D], BF16, kind="ExternalInput")
    wkT = nc.dram_tensor("wkT", [L_layers, D, D], BF16, kind="ExternalInput")
    wvT = nc.dram_tensor("wvT", [L_layers, D, D], BF16, kind="ExternalInput")
    bqkvr = nc.dram_tensor("bqkvr", [L_layers, 3, 1, D], BF16,
                           kind="ExternalInput")
    w1T = nc.dram_tensor("w1T", [L_layers, D, DF], BF16, kind="ExternalInput")
    b1c = nc.dram_tensor("b1c", [L_layers, P, NDF], F32, kind="ExternalInput")
    w2T = nc.dram_tensor("w2T", [L_layers, DF, D], BF16, kind="ExternalInput")
    b2c = nc.dram_tensor("b2c", [L_layers, P, NF], F32, kind="ExternalInput")
    lng1 = nc.dram_tensor("lng1", [L_layers, P, NF], F32, kind="ExternalInput")
    lnb1 = nc.dram_tensor("lnb1", [L_layers, P, NF], F32, kind="ExternalInput")
    lng2 = nc.dram_tensor("lng2", [L_layers, P, NF], F32, kind="ExternalInput")
    lnb2 = nc.dram_tensor("lnb2", [L_layers, P, NF], F32, kind="ExternalInput")
    selc = nc.dram_tensor("selc", [3, 2, P], F32, kind="ExternalInput")
    out = nc.dram_tensor("out", [S, D], F32, kind="ExternalOutput")

    with tile.TileContext(nc) as tc, ExitStack() as ctx:
        # ---- pools --------------------------------------------------------
        xp = ctx.enter_context(tc.tile_pool(name="xp", bufs=6))      # [128,1024] f32 residual master
        xbp = ctx.enter_context(tc.tile_pool(name="xbp", bufs=6))    # [128,1024] bf16 matmul copy of x
        bfp = ctx.enter_context(tc.tile_pool(name="bfp", bufs=30))   # [128,1024] bf16 shared ring
        qtp = ctx.enter_context(tc.tile_pool(name="qtp", bufs=4))    # [128,1032] bf16 qT/kT pairs
        ptp = ctx.enter_context(tc.tile_pool(name="ptp", bufs=3))    # [128,1024] bf16 exp(scores)
        vap = ctx.enter_context(tc.tile_pool(name="vap", bufs=4))    # [128,520] bf16 v-aug
        f32s = ctx.enter_context(tc.tile_pool(name="f32s", bufs=3))  # f32 scratch (tags)
        wqp = ctx.enter_context(tc.tile_pool(name="wqp", bufs=8))    # [128,768] bf16 qkv weights
        w1p = ctx.enter_context(tc.tile_pool(name="w1p", bufs=6))    # [128,1536] bf16 ffn1 weights
        w2p = ctx.enter_context(tc.tile_pool(name="w2p", bufs=48))   # [128,128] bf16 ffn2 weight chunks
        smp = ctx.enter_context(tc.tile_pool(name="smp", bufs=8))    # small tiles
        cst = ctx.enter_context(tc.tile_pool(name="cst", bufs=1))
        drp = ctx.enter_context(tc.tile_pool(name="drp", bufs=1, space="DRAM"))

        ps_big = ctx.enter_context(tc.tile_pool(name="ps_big", bufs=2, space="PSUM"))
        ps_av = ctx.enter_context(tc.tile_pool(name="ps_av", bufs=2, space="PSUM"))

        # ---- constants ----------------------------------------------------
        idb = cst.tile([P, P], BF16)
        make_identity(nc, idb)
        ones_row = cst.tile([1, P], BF16)
        nc.vector.memset(ones_row[:], 1.0)
        ones_col = cst.tile([P, 1], BF16)
        nc.vector.memset(ones_col[:], 1.0)
        eps_c = cst.tile([P, 1], F32)
        nc.vector.memset(eps_c[:], EPS)
        # broadcast helpers: out[128, N] = sel.T @ rows  (PE-based broadcast)
        sel2 = cst.tile([2, P], F32)
        nc.sync.dma_start(sel2[:], selc[0])
        selr0 = cst.tile([2, P], F32)
        nc.sync.dma_start(selr0[:], selc[1])
        selr1 = cst.tile([2, P], F32)
        nc.sync.dma_start(selr1[:], selc[2])

        # DRAM scratch for q/k/v in flat (S*12, 64) layout: head h of the
        # torch .view() is rows [1024h, 1024h+1024) of the flat matrix.
        qkv_dram = drp.tile([3, S * H, HD], BF16)

        # ---- embedding: x = emb[ids] + pe, transposed to feature-major ----
        x = [xp.tile([P, S], F32, tag="x", name=f"x{F}") for F in range(NF)]
        xbf = [xbp.tile([P, S], BF16, tag="xbf", name=f"xbf{F}")
               for F in range(NF)]
        for T in range(NT):
            idt = smp.tile([P, 1], I32, tag="idt")
            nc.sync.dma_start(idt[:], ids[P * T:P * (T + 1), :])
            g = f32s.tile([P, D], F32, tag="sb")
            nc.gpsimd.indirect_dma_start(
                out=g[:], out_offset=None, in_=emb[:],
                in_offset=bass.IndirectOffsetOnAxis(ap=idt[:, :1], axis=0))
            pet = f32s.tile([P, D], F32, tag="sb")
            nc.sync.dma_start(pet[:], pe[P * T:P * (T + 1), :])
            xt = f32s.tile([P, D], F32, tag="sb")
            nc.vector.tensor_add(xt[:], g[:], pet[:])
            xtb = bfp.tile([P, D], BF16, tag="bf")
            nc.scalar.copy(xtb[:], xt[:])
            for F in range(NF):
                tr = ps_big.tile([P, P], BF16, tag="big")
                nc.tensor.transpose(tr[:], xtb[:, P * F:P * (F + 1)], idb[:])
                nc.vector.tensor_copy(xbf[F][:, P * T:P * (T + 1)], tr[:])
        for F in range(NF):
            nc.vector.tensor_copy(x[F][:], xbf[F][:])

        def dump_fm(tiles, stage):
            """Debug: DMA up to 6 [128, <=1024] tiles into `out` (flat)."""
            if debug_stage != stage:
                return
            flat = out[:].rearrange("s d -> (s d)").rearrange(
                "(f p t) -> f p t", f=6, p=P)
            for i, t in enumerate(tiles[:6]):
                tt = t
                if t.dtype != F32:
                    ft = f32s.tile([P, S], F32, tag="dump", bufs=2)
                    nc.vector.tensor_copy(ft[0:t.shape[0], 0:t.free_size()],
                                          t[:])
                    tt = ft[0:t.shape[0], 0:t.free_size()]
                nc.sync.dma_start(flat[i, 0:tt.shape[0], 0:1024]
                                  if tt.free_size() >= 1024 else
                                  flat[i, 0:tt.shape[0], 0:tt.free_size()],
                                  tt[:, 0:1024] if tt.free_size() >= 1024
                                  else tt[:])

        dump_fm(x, "embed")

        for l in range(L_layers):
            # ---- per-layer consts ----------------------------------------
            g1t = smp.tile([P, NF], F32, tag="ln")
            nc.sync.dma_start(g1t[:], lng1[l])
            b1t = smp.tile([P, NF], F32, tag="ln")
            nc.sync.dma_start(b1t[:], lnb1[l])
            g2t = smp.tile([P, NF], F32, tag="ln")
            nc.sync.dma_start(g2t[:], lng2[l])
            b2t = smp.tile([P, NF], F32, tag="ln")
            nc.sync.dma_start(b2t[:], lnb2[l])
            b1ct = smp.tile([P, NDF], F32, tag="b1")
            nc.sync.dma_start(b1ct[:], b1c[l])
            b2ct = smp.tile([P, NF], F32, tag="b2")
            nc.sync.dma_start(b2ct[:], b2c[l])

            # ---- Q/K/V projections, token-major -> DRAM flat -------------
            for ti, wT in enumerate((wqT, wkT, wvT)):
                wsb = []
                for F in range(NF):
                    w = wqp.tile([P, D], BF16, tag="w")
                    nc.sync.dma_start(w[:], wT[l, P * F:P * (F + 1), :])
                    wsb.append(w)
                brow = smp.tile([1, D], BF16, tag="brow", bufs=3)
                nc.sync.dma_start(brow[:], bqkvr[l, ti])
                for T in range(NT):
                    ps = ps_big.tile([P, S], F32, tag="big")
                    for nb, (n0, n1) in enumerate(((0, 512), (512, 768))):
                        for F in range(NF):
                            nc.tensor.matmul(
                                ps[:, n0:n1],
                                lhsT=xbf[F][:, P * T:P * (T + 1)],
                                rhs=wsb[F][:, n0:n1],
                                start=(F == 0), stop=False)
                        nc.tensor.matmul(
                            ps[:, n0:n1], lhsT=ones_row[:, :],
                            rhs=brow[:, n0:n1], start=False, stop=True)
                    ev = bfp.tile([P, D], BF16, tag="bf")
                    nc.vector.tensor_copy(ev[:], ps[:, :D])
                    nc.sync.dma_start(
                        qkv_dram[ti, H * P * T: H * P * (T + 1), :]
                        .rearrange("(p a) d -> p a d", p=P),
                        ev[:].rearrange("p (a d) -> p a d", d=HD))

            # ---- attention, head pairs (A=2p at part 0-63, B at 64-127) --
            for p in range(H // 2):
                hA, hB = 2 * p, 2 * p + 1
                va2 = []
                for h in (hA, hB):
                    va = vap.tile([P, KT * (HD + 1)], BF16, tag="va", bufs=3)
                    nc.sync.dma_start(
                        va[:].rearrange("p (k d) -> p k d", d=HD + 1)[:, :, 0:HD],
                        qkv_dram[2, S * h:S * (h + 1), :]
                        .rearrange("(k p) d -> p k d", p=P))
                    nc.vector.memset(
                        va[:].rearrange("p (k d) -> p k d", d=HD + 1)
                        [:, :, HD:HD + 1], 1.0)
                    va2.append(va)

                # qT/kT [128, 1024] for the pair: head h is rows
                # [1024h, 1024h+1024) of the flat (S*12, 64) matrix.  Load
                # token-major (va pattern) and PE-transpose 8 chunks per
                # half; half B transposes land on PSUM partitions 64-127
                # via tile_position col offset.  One DVE copy per tile.
                qT = qtp.tile([HD * 2, S], BF16, tag="qt")
                kTt = qtp.tile([HD * 2, S], BF16, tag="qt")
                for ti, dst in ((0, qT), (1, kTt)):
                    trp = ps_big.tile([P, S], BF16, tag="big")
                    for half, h in ((0, hA), (1, hB)):
                        qa = vap.tile([P, KT * HD], BF16, tag="qa", bufs=3)
                        nc.sync.dma_start(
                            qa[:].rearrange("p (k d) -> p k d", d=HD),
                            qkv_dram[ti, S * h:S * (h + 1), :]
                            .rearrange("(k p) d -> p k d", p=P))
                        for k in range(KT):
                            nc.tensor.transpose(
                                trp[HD * half:HD * (half + 1),
                                    P * k:P * (k + 1)],
                                qa[:, HD * k:HD * (k + 1)], idb[:])
                    nc.vector.tensor_copy(dst[:], trp[:])
                avA = ps_av.tile([HD + 1, S], F32, tag="av")
                avB = ps_av.tile([HD + 1, S], F32, tag="av")
                for k in range(KT):
                    for nb in range(2):
                        sc = ps_big.tile([P, S], F32, tag="big")
                        nc.tensor.matmul(
                            sc[:, 0:512],
                            lhsT=kTt[0:HD, P * k:P * (k + 1)],
                            rhs=qT[0:HD, 512 * nb:512 * (nb + 1)],
                            start=True, stop=True)
                        nc.tensor.matmul(
                            sc[:, 512:1024],
                            lhsT=kTt[HD:P, P * k:P * (k + 1)],
                            rhs=qT[HD:P, 512 * nb:512 * (nb + 1)],
                            start=True, stop=True)
                        pt = ptp.tile([P, S], BF16, tag="pt")
                        nc.scalar.activation(pt[:], sc[:], AF.Exp,
                                             scale=1.0 / 8.0)
                        if l == 0 and p == 0 and k == 0 and nb == 0:
                            dump_fm([pt], "exp00")
                        nc.tensor.matmul(
                            avA[:, 512 * nb:512 * (nb + 1)],
                            lhsT=va2[0][:, (HD + 1) * k:(HD + 1) * (k + 1)],
                            rhs=pt[:, 0:512],
                            start=(k == 0), stop=(k == KT - 1))
                        nc.tensor.matmul(
                            avB[:, 512 * nb:512 * (nb + 1)],
                            lhsT=va2[1][:, (HD + 1) * k:(HD + 1) * (k + 1)],
                            rhs=pt[:, 512:1024],
                            start=(k == 0), stop=(k == KT - 1))

                # normalize and accumulate into the residual, feature-major:
                # head A -> x[p][0:64], head B -> x[p][64:128].
                sbA = f32s.tile([HD + 1, S], F32, tag="sb")
                nc.vector.tensor_copy(sbA[:], avA[:])
                sbB = f32s.tile([HD + 1, S], F32, tag="sb")
                nc.vector.tensor_copy(sbB[:], avB[:])
                nc.vector.reciprocal(sbA[HD:HD + 1, :], sbA[HD:HD + 1, :])
                nc.vector.reciprocal(sbB[HD:HD + 1, :], sbB[HD:HD + 1, :])
                sbBsh = f32s.tile([P, S], F32, tag="sb")
                nc.sync.dma_start(sbBsh[HD:P, :], sbB[0:HD, :])
                recs = f32s.tile([2, S], F32, tag="rc", bufs=2)
                nc.sync.dma_start(recs[0:1, :], sbA[HD:HD + 1, :])
                nc.sync.dma_start(recs[1:2, :], sbB[HD:HD + 1, :])
                bk = ps_big.tile([P, S], F32, tag="big")
                for nb in range(2):
                    nc.tensor.matmul(bk[:, 512 * nb:512 * (nb + 1)],
                                     lhsT=sel2[:, :],
                                     rhs=recs[:, 512 * nb:512 * (nb + 1)],
                                     start=True, stop=True)
                if l == 0 and p == 0:
                    dump_fm([sbA, sbB, bk, sbBsh], "norm0")
                nc.vector.tensor_tensor(sbA[0:HD, :], sbA[0:HD, :],
                                        bk[0:HD, :], op=OP.mult)
                nc.vector.tensor_tensor(sbBsh[HD:P, :], sbBsh[HD:P, :],
                                        bk[HD:P, :], op=OP.mult)
                nc.vector.tensor_add(x[p][0:HD, :], x[p][0:HD, :],
                                     sbA[0:HD, :])
                nc.vector.tensor_add(x[p][HD:P, :], x[p][HD:P, :],
                                     sbBsh[HD:P, :])
            if l == 0:
                dump_fm(x, "attn")

            # ---- feature-major LayerNorm (in place on x) -----------------
            def layernorm_fm(g_t, b_t, out_tag, out_name):
                st = ps_av.tile([33, S], F32, tag="av")
                for F in range(NF):
                    rb = bfp.tile([P, S], BF16, tag="bf")
                    nc.vector.tensor_copy(rb[:], x[F][:])
                    sq = bfp.tile([P, S], BF16, tag="bf")
                    nc.vector.tensor_tensor(sq[:], x[F][:], x[F][:],
                                            op=OP.mult)
                    for nb in range(2):
                        nc.tensor.matmul(
                            st[0:1, 512 * nb:512 * (nb + 1)],
                            lhsT=ones_col[:, :],
                            rhs=rb[:, 512 * nb:512 * (nb + 1)],
                            start=(F == 0), stop=(F == NF - 1))
                        nc.tensor.matmul(
                            st[32:33, 512 * nb:512 * (nb + 1)],
                            lhsT=ones_col[:, :],
                            rhs=sq[:, 512 * nb:512 * (nb + 1)],
                            start=(F == 0), stop=(F == NF - 1))
                sts = f32s.tile([33, S], F32, tag="sb")
                nc.vector.tensor_copy(sts[32:33, :], st[32:33, :])
                strows = f32s.tile([2, S], F32, tag="rc", bufs=2)
                nc.vector.tensor_copy(strows[0:1, :], st[0:1, :])
                nc.sync.dma_start(strows[1:2, :], sts[32:33, :])
                mS_ps = ps_big.tile([P, S], F32, tag="big")
                qS_ps = ps_big.tile([P, S], F32, tag="big")
                for nb in range(2):
                    nc.tensor.matmul(mS_ps[:, 512 * nb:512 * (nb + 1)],
                                     lhsT=selr0[:, :],
                                     rhs=strows[:, 512 * nb:512 * (nb + 1)],
                                     start=True, stop=True)
                    nc.tensor.matmul(qS_ps[:, 512 * nb:512 * (nb + 1)],
                                     lhsT=selr1[:, :],
                                     rhs=strows[:, 512 * nb:512 * (nb + 1)],
                                     start=True, stop=True)
                # mean, var, rstd, -mean*rstd  (all [128, S] broadcast tiles)
                mS = f32s.tile([P, S], F32, tag="bk", bufs=2)
                nc.vector.tensor_scalar_mul(mS[:], mS_ps[:], 1.0 / D)
                qS = f32s.tile([P, S], F32, tag="lns")
                nc.vector.tensor_scalar_mul(qS[:], qS_ps[:], 1.0 / D)
                msq = f32s.tile([P, S], F32, tag="lns")
                nc.vector.tensor_tensor(msq[:], mS[:], mS[:], op=OP.mult)
                nc.vector.tensor_tensor(qS[:], qS[:], msq[:], op=OP.subtract)
                sd = msq  # reuse storage
                nc.scalar.activation(sd[:], qS[:], AF.Sqrt, bias=eps_c[:, :1])
                rstd = qS  # reuse storage
                nc.vector.reciprocal(rstd[:], sd[:])
                nmr = f32s.tile([P, S], F32, tag="lns")
                nc.vector.tensor_tensor(nmr[:], mS[:], rstd[:], op=OP.mult)
                nc.vector.tensor_scalar_mul(nmr[:], nmr[:], -1.0)
                outs = []
                for F in range(NF):
                    cF = f32s.tile([P, S], F32, tag="sb")
                    nc.vector.tensor_scalar(cF[:], nmr[:], g_t[:, F:F + 1],
                                            b_t[:, F:F + 1],
                                            op0=OP.mult, op1=OP.add)
                    t = f32s.tile([P, S], F32, tag="sb")
                    nc.vector.tensor_tensor(t[:], x[F][:], rstd[:], op=OP.mult)
                    nc.vector.scalar_tensor_tensor(
                        x[F][:], t[:], g_t[:, F:F + 1], cF[:],
                        op0=OP.mult, op1=OP.add)
                    ob = (bfp.tile([P, S], BF16, tag="bf",
                                   name=f"{out_name}{F}")
                          if out_tag == "bf" else
                          xbp.tile([P, S], BF16, tag="xbf",
                                   name=f"{out_name}{F}"))
                    nc.vector.tensor_copy(ob[:], x[F][:])
                    outs.append(ob)
                return outs

            y1bf = layernorm_fm(g1t, b1t, "bf", "y1bf")
            if l == 0:
                dump_fm(x, "ln1")

            # ---- FFN (feature-major) -------------------------------------
            hT = []
            for half in range(2):
                w1sb = []
                for F in range(NF):
                    w = w1p.tile([P, DF // 2], BF16, tag="w1")
                    nc.sync.dma_start(
                        w[:], w1T[l, P * F:P * (F + 1),
                                  (DF // 2) * half:(DF // 2) * (half + 1)])
                    w1sb.append(w)
                for ci in range(NDF // 2):
                    c = (NDF // 2) * half + ci
                    ps = ps_big.tile([P, S], F32, tag="big")
                    for F in range(NF):
                        for nb in range(2):
                            nc.tensor.matmul(
                                ps[:, 512 * nb:512 * (nb + 1)],
                                lhsT=w1sb[F][:, P * ci:P * (ci + 1)],
                                rhs=y1bf[F][:, 512 * nb:512 * (nb + 1)],
                                start=(F == 0), stop=(F == NF - 1))
                    ht = bfp.tile([P, S], BF16, tag="bf", name=f"ht{c}")
                    nc.scalar.activation(ht[:], ps[:], AF.Relu,
                                         bias=b1ct[:, c:c + 1])
                    hT.append(ht)
            for j in range(NF):
                w2sb = []
                for c in range(NDF):
                    w = w2p.tile([P, P], BF16, tag="w2")
                    nc.sync.dma_start(
                        w[:], w2T[l, P * c:P * (c + 1), P * j:P * (j + 1)])
                    w2sb.append(w)
                ps = ps_big.tile([P, S], F32, tag="big")
                for c in range(NDF):
                    for nb in range(2):
                        nc.tensor.matmul(
                            ps[:, 512 * nb:512 * (nb + 1)],
                            lhsT=w2sb[c][:, :],
                            rhs=hT[c][:, 512 * nb:512 * (nb + 1)],
                            start=(c == 0), stop=(c == NDF - 1))
                f2 = f32s.tile([P, S], F32, tag="sb")
                nc.vector.tensor_scalar_add(f2[:], ps[:], b2ct[:, j:j + 1])
                nc.vector.tensor_add(x[j][:], x[j][:], f2[:])
            if l == 0:
                dump_fm(x, "ffn")

            # ---- LN2 -> x (and bf16 copies become next layer's xbf) ------
            xbf = layernorm_fm(g2t, b2t, "xbf", "xnbf")

        # ---- output: fm -> token-major ------------------------------------
        for F in range(NF if debug_stage is None else 0):
            for T in range(NT):
                tr = ps_big.tile([P, P], BF16, tag="big")
                nc.tensor.transpose(tr[:], xbf[F][:, P * T:P * (T + 1)],
                                    idb[:])
                of = f32s.tile([P, P], F32, tag="sb")
                nc.vector.tensor_copy(of[:], tr[:])
                nc.sync.dma_start(out[P * T:P * (T + 1), P * F:P * (F + 1)],
                                  of[:])

    nc.compile()
    return nc


def _selc():
    s = np.zeros((3, 2, P), np.float32)
    s[0, 0, 0:HD] = 1.0   # sel2: head-A rows from stat row 0
    s[0, 1, HD:P] = 1.0   # sel2: head-B rows from stat row 1
    s[1, 0, :] = 1.0      # selr0: broadcast row 0
    s[2, 1, :] = 1.0      # selr1: broadcast row 1
    return s


def _prep_in_maps(inputs):
    bf = ml_dtypes.bfloat16
    Lw = np.asarray(inputs["Wq"]).shape[0]

    def fmc(v):  # [L, D] -> [L, P, NF] feature-chunk-major
        return np.ascontiguousarray(
            np.asarray(v, np.float32).reshape(Lw, NF, P).transpose(0, 2, 1))

    shared = {
        "emb": np.ascontiguousarray(np.asarray(inputs["emb"], np.float32)),
        "pe": np.ascontiguousarray(np.asarray(inputs["pe"], np.float32)[:S]),
        "wqT": np.ascontiguousarray(
            np.asarray(inputs["Wq"]).transpose(0, 2, 1)).astype(bf),
        "wkT": np.ascontiguousarray(
            np.asarray(inputs["Wk"]).transpose(0, 2, 1)).astype(bf),
        "wvT": np.ascontiguousarray(
            np.asarray(inputs["Wv"]).transpose(0, 2, 1)).astype(bf),
        "bqkvr": np.ascontiguousarray(np.stack(
            [np.asarray(inputs["bq"]), np.asarray(inputs["bk"]),
             np.asarray(inputs["bv"])], axis=1)
            .reshape(Lw, 3, 1, D)).astype(bf),
        "w1T": np.ascontiguousarray(
            np.asarray(inputs["W1"]).transpose(0, 2, 1)).astype(bf),
        "b1c": np.ascontiguousarray(
            np.asarray(inputs["b1"], np.float32).reshape(Lw, NDF, P)
            .transpose(0, 2, 1)),
        "w2T": np.ascontiguousarray(
            np.asarray(inputs["W2"]).transpose(0, 2, 1)).astype(bf),
        "b2c": fmc(inputs["b2"]),
        "lng1": fmc(inputs["ln1_g"]),
        "lnb1": fmc(inputs["ln1_b"]),
        "lng2": fmc(inputs["ln2_g"]),
        "lnb2": fmc(inputs["ln2_b"]),
        "selc": _selc(),
    }
    ids_all = np.asarray(inputs["input_ids"]).astype(np.int32)
    in_maps = []
    for c in range(N_CORES):
        m = dict(shared)
        m["ids"] = np.ascontiguousarray(ids_all[c].reshape(S, 1))
        in_maps.append(m)
    return in_maps


_CACHE = {}


def _get_runner():
    """Build (once) the compiled NEFF wrapped in a sharded PJRT callable."""
    if "run" in _CACHE:
        return _CACHE["run"]
    _lazy_imports()
    mybir = _BASS["mybir"]
    import jax
    import jax.numpy as jnp
    from jax.sharding import Mesh, PartitionSpec
    from jax.experimental.shard_map import shard_map
    from concourse import bass2jax
    from concourse.bass2jax import _bass_exec_p, partition_id_tensor

    nc = _build_encoder()
    bass2jax.install_neuronx_cc_hook()

    in_names, out_names, out_avals, zero_outs = [], [], [], []
    partition_name = (nc.partition_id_tensor.name
                      if nc.partition_id_tensor else None)
    for alloc in nc.m.functions[0].allocations:
        if not isinstance(alloc, mybir.MemoryLocationSet):
            continue
        name = alloc.memorylocations[0].name
        if alloc.kind == "ExternalInput":
            if name != partition_name:
                in_names.append(name)
        elif alloc.kind == "ExternalOutput":
            out_names.append(name)
            shape = tuple(alloc.tensor_shape)
            dtype = mybir.dt.np(alloc.dtype)
            out_avals.append(jax.core.ShapedArray(shape, dtype))
            zero_outs.append(np.zeros(shape, dtype))
    n_params = len(in_names)
    n_outs = len(out_avals)
    all_in_names = list(in_names) + list(out_names)
    if partition_name is not None:
        all_in_names.append(partition_name)
    donate = tuple(range(n_params, n_params + n_outs))

    def _body(*args):
        operands = list(args)
        if partition_name is not None:
            operands.append(partition_id_tensor())
        outs = _bass_exec_p.bind(
            *operands,
            out_avals=tuple(out_avals),
            in_names=tuple(all_in_names),
            out_names=tuple(out_names),
            lowering_input_output_aliases=(),
            sim_require_finite=True,
            sim_require_nnan=True,
            nc=nc,
        )
        return tuple(outs)

    devices = jax.devices()[:N_CORES]
    mesh = Mesh(np.asarray(devices), ("core",))
    in_specs = (PartitionSpec("core"),) * (n_params + n_outs)
    out_specs = (PartitionSpec("core"),) * n_outs
    sharded = jax.jit(
        shard_map(_body, mesh=mesh, in_specs=in_specs, out_specs=out_specs,
                  check_rep=False),
        donate_argnums=donate, keep_unused=True)

    def run(in_maps, timing_iters=0):
        concat_in = [
            np.concatenate([np.asarray(in_maps[c][k]) for c in range(N_CORES)],
                           axis=0)
            for k in in_names
        ]
        concat_zeros = [
            np.zeros((N_CORES * z.shape[0], *z.shape[1:]), z.dtype)
            for z in zero_outs
        ]
        out_arrs = sharded(*concat_in, *concat_zeros)
        results = [
            {name: np.asarray(out_arrs[i]).reshape(
                N_CORES, *out_avals[i].shape)[c]
             for i, name in enumerate(out_names)}
            for c in range(N_CORES)
        ]
        times = []
        if timing_iters:
            import time
            stage = jax.jit(shard_map(
                lambda *xs: xs, mesh=mesh,
                in_specs=(PartitionSpec("core"),) * len(concat_in),
                out_specs=(PartitionSpec("core"),) * len(concat_in),
                check_rep=False))
            dev_in = stage(*concat_in)
            jax.block_until_ready(dev_in)
            for _ in range(timing_iters):
                zs = [z.copy() for z in concat_zeros]
                t0 = time.perf_counter()
                o = sharded(*dev_in, *zs)
                jax.block_until_ready(o)
                times.append(time.perf_counter() - t0)
        return results, times

    _CACHE["run"] = run
    return run


def kernel(**inputs):
    run = _get_runner()
    in_maps = _prep_in_maps(inputs)
    results, _ = run(in_maps)
    out = np.stack([results[c]["out"] for c in range(N_CORES)], axis=0)
    return out
